# revision 32
# baseline (speedup 1.0000x reference)
"""Trainium2 Bass kernel for BatchRankingLoss — sorted-prefix single-matmul design.

Reference (B=131072, d=256, K=512 complexes, G=511 groups):
    dt = t_i - t_j ; w = |dt| > 0.1 ; y = sign-ish(dt)
    dL = w * max(0, 1 + y*(o_i - o_j)) ; loss = sum(dL) / (G*d*(d-1))

Identity: dL is symmetric in (i,j) among active pairs, so
    sum(dL) = 2 * sum_{(i,j): t_j < t_i - 0.1} relu((1 + o_i) - o_j)

The host sorts each group's decoys by t, so the active j's for row i are
exactly the prefix j < c_i (c_i = #{j: t_j < t_i - 0.1}, non-decreasing
in i). This removes both the threshold-mask computation and the u-matmul
entirely: the device only evaluates relu((1+o_i) - o_j) over data-tight
prefix extents, and the handful of over-included boundary pairs (all
inactive, |dt| <= 0.1) are subtracted exactly on the host.

Device layout per core (64 groups):
  partition p = (g_local = p//2, parity = p%2)
  slice s in [0,32): islot k in [0,4) covers decoy i = 8s + 2k + parity
  free axis interleaved: col = 4j + k, live prefix [0, 4*J_s),
  J_s = data-tight max c over the slice's rows (global over cores,
  rounded up to 2; compiled program cached per J-schedule).

  One bf16 matmul per slice (contraction 68 = 64 group one-hot rows +
  4 islot a-rows, a = 1 + o_i) produces do = (1+o_i) - o_j in fp32 PSUM.
  Slice extents are packed column-continuously into [128, 1024] 2-bank
  PSUM tiles (bufs=4) so every tile fills completely; each tile gets ONE
  relu+accumulate pass — ScalarE activation(Relu, accum_out) or VectorE
  tensor_scalar(max, 0, accum-add) — assigned by a greedy time balance
  (ACT ~= 373ns + fill/1.2GHz, DVE ~= 125ns + fill/0.96GHz). ScalarE and
  VectorE drain different PSUM tiles concurrently; PE fills two tiles
  ahead. Weights stream in chunks across the SP and SWDGE DMA queues in
  first-use order; all-but-last accumulator columns DMA out early so only
  one column's latency rides the tail.

Padded rows (group 511) use a = -1000 so they contribute exactly 0.
"""

import numpy as np
from contextlib import ExitStack

import concourse.bacc as bacc
import concourse.mybir as mybir
import concourse.tile as tile
from concourse.bass_utils import run_bass_kernel_spmd

import ml_dtypes

N_CORES = 8
D = 256
G_REAL = 511
G_PAD = 512
GPC = G_PAD // N_CORES   # 64 groups per core
P = 128                  # partitions: p = 2*g_local + parity
M = 4                    # islots per slice
WIN = 2 * M              # i-window per slice
N_SLICES = D // WIN      # 32
KDIM = GPC + M           # matmul contraction rows
RHS_W = M * D            # rhs moving-operand width
N_PAIRS = G_REAL * D * (D - 1)
PSUM_COLS = 1024         # fp32 cols per PSUM tile (2 banks)
PSUM_BUFS = 4
BANK = 512               # fp32 cols per PSUM bank (matmul piece limit)

THRESHOLD = np.float32(0.1)
PAD_A = np.float32(-1000.0)

_CACHED = {}


def _schedule(J):
    """Column-continuous packing: slices may straddle PSUM-tile boundaries
    so every tile fills to PSUM_COLS. Returns [(fill, [(s, off, lo, hi)])]
    where the piece covers rhs cols [lo, hi) of slice s at tile offset off."""
    tiles = []
    cur = []
    fill = 0
    for s in range(N_SLICES):
        E = M * int(J[s])
        lo = 0
        while lo < E:
            take = min(E - lo, PSUM_COLS - fill)
            cur.append((s, fill, lo, lo + take))
            fill += take
            lo += take
            if fill == PSUM_COLS:
                tiles.append((fill, cur))
                cur, fill = [], 0
    if cur:
        tiles.append((fill, cur))
    return tiles


def _assign_engines(tiles):
    """Exact 2-engine makespan minimization over tile->engine assignments.
    Costs are sim-fit: ACT ~373ns/op + 1.2G/s, DVE ~125ns/op + 0.96G/s.
    Interleaved assignments are preferred (tie-break) so consecutive tiles
    tend to alternate engines for pipeline overlap."""
    n = len(tiles)
    fills = [f for f, _ in tiles]
    if n > 20:  # fall back to greedy for pathological schedules
        t_dve = t_act = 0.0
        out = []
        for fill in fills:
            if t_dve + 125.0 + fill / 0.959 <= t_act + 373.0 + fill / 1.202:
                out.append("dve")
                t_dve += 125.0 + fill / 0.959
            else:
                out.append("act")
                t_act += 373.0 + fill / 1.202
        return out
    best = None
    for mask in range(1 << n):
        t_act = t_dve = 0.0
        for i in range(n):
            if mask >> i & 1:
                t_act += 373.0 + fills[i] / 1.202
            else:
                t_dve += 125.0 + fills[i] / 0.959
        # penalize same-engine runs a little to keep the pipeline alternating
        runs = sum(1 for i in range(n - 1)
                   if (mask >> i & 1) == (mask >> (i + 1) & 1))
        score = max(t_act, t_dve) + 15.0 * runs
        if best is None or score < best[0]:
            best = (score, mask)
    mask = best[1]
    return ["act" if mask >> i & 1 else "dve" for i in range(n)]


def _build_program(J, repeat=1, mode="full"):
    J = tuple(int(x) for x in J)
    tiles = _schedule(J)
    engines = _assign_engines(tiles)
    n_act = max(sum(1 for e in engines if e == "act"), 1)
    n_dve = max(sum(1 for e in engines if e == "dve"), 1)

    assert sum(1 for e in engines if e == "act") >= 2
    assert sum(1 for e in engines if e == "dve") >= 2
    nc = bacc.Bacc("TRN2", target_bir_lowering=False, debug=False,
                   num_devices=N_CORES)
    f32 = mybir.dt.float32
    bf16 = mybir.dt.bfloat16

    active = [s for s in range(N_SLICES) if J[s] > 0]
    CS = 8
    chunks = [active[i:i + CS] for i in range(0, len(active), CS)]
    chunk_of = {}
    for ch, sl in enumerate(chunks):
        for pos, s in enumerate(sl):
            chunk_of[s] = (ch, pos)

    w_ds = [nc.dram_tensor(f"w{ch}", [KDIM, len(sl) * P], bf16,
                           kind="ExternalInput")
            for ch, sl in enumerate(chunks)]
    rhs_d = nc.dram_tensor("rhs", [KDIM, RHS_W], bf16, kind="ExternalInput")
    acc_a_d = nc.dram_tensor("acc_a", [P, n_act - 1], f32,
                             kind="ExternalOutput")
    acc_d_d = nc.dram_tensor("acc_d", [P, n_dve - 1], f32,
                             kind="ExternalOutput")
    accl_a_d = nc.dram_tensor("accl_a", [P, 1], f32, kind="ExternalOutput")
    accl_d_d = nc.dram_tensor("accl_d", [P, 1], f32, kind="ExternalOutput")

    with ExitStack() as ctx:
        tc = ctx.enter_context(tile.TileContext(nc, num_cores=N_CORES))
        consts = ctx.enter_context(tc.tile_pool(name="consts", bufs=1))
        psum = ctx.enter_context(
            tc.tile_pool(name="ps", bufs=PSUM_BUFS, space="PSUM"))
        h_dve = ctx.enter_context(tc.tile_pool(name="hd", bufs=2))
        h_act = ctx.enter_context(tc.tile_pool(name="ha", bufs=2))

        w_c = []
        for ch, sl in enumerate(chunks):
            w_ci = consts.tile([KDIM, len(sl) * P], bf16, name=f"w_c{ch}")
            w_c.append(w_ci)
        rhs_t = consts.tile([KDIM, RHS_W], bf16)
        acc_a = consts.tile([P, n_act - 1], f32)
        acc_d = consts.tile([P, n_dve - 1], f32)
        accl_a = consts.tile([P, 1], f32)
        accl_d = consts.tile([P, 1], f32)

        # Input DMAs: only w_c0 on the SP queue — the first matmuls' wait
        # coalesces over ~3 tiles of matmuls, so SP must not carry a second
        # transfer. rhs + the later chunks ride the gpsimd SWDGE queue in
        # need-order. ACT's stream stays free so its hoisted
        # activation-table load is its only pre-pipeline work.
        nc.gpsimd.dma_start(rhs_t[:], rhs_d[:])
        for ch in range(len(chunks)):
            eng = nc.sync if ch == 0 else nc.gpsimd
            eng.dma_start(w_c[ch][:], w_ds[ch][:])

        def lhsT_of(s):
            ch, pos = chunk_of[s]
            return w_c[ch][:, pos * P:(pos + 1) * P]

        ia = idv = 0
        for rep in range(repeat):
            ia = idv = 0
            for (fill, slices), eng in zip(tiles, engines):
                ps = psum.tile([P, PSUM_COLS], f32, tag="ps")
                for (s, off, lo, hi) in slices:
                    a = off
                    while a < off + (hi - lo):
                        b = min((a // BANK + 1) * BANK, off + (hi - lo))
                        nc.tensor.matmul(
                            ps[:, a:b],
                            lhsT=lhsT_of(s),
                            rhs=rhs_t[:, lo + (a - off):lo + (b - off)],
                            start=True, stop=True,
                        )
                        a = b
                if mode == "mm":
                    continue
                if eng == "dve":
                    h = h_dve.tile([P, PSUM_COLS], f32, tag="hd")
                    last = idv == n_dve - 1
                    nc.vector.tensor_scalar(
                        out=h[:, 0:fill], in0=ps[:, 0:fill],
                        scalar1=0.0, scalar2=None,
                        op0=mybir.AluOpType.max,
                        op1=mybir.AluOpType.add,
                        accum_out=(accl_d[:, 0:1] if last
                                   else acc_d[:, idv:idv + 1]),
                    )
                    if idv == n_dve - 2 and mode == "full" and rep == repeat - 1:
                        nc.sync.dma_start(acc_d_d[:], acc_d[:])
                    idv += 1
                else:
                    h = h_act.tile([P, PSUM_COLS], f32, tag="ha")
                    last = ia == n_act - 1
                    nc.scalar.activation(
                        h[:, 0:fill], ps[:, 0:fill],
                        mybir.ActivationFunctionType.Relu,
                        accum_out=(accl_a[:, 0:1] if last
                                   else acc_a[:, ia:ia + 1]),
                    )
                    if ia == n_act - 2 and mode == "full" and rep == repeat - 1:
                        nc.sync.dma_start(acc_a_d[:], acc_a[:])
                    ia += 1

        if mode == "full":
            # only the last column of each engine rides the critical tail
            nc.scalar.dma_start(accl_a_d[:], accl_a[:])
            nc.sync.dma_start(accl_d_d[:], accl_d[:])

    nc.compile()
    return nc, tiles


def _host_prep(input, gdt_ts):
    """Sort groups by t, compute prefix counts c, extents J, per-core input
    arrays, and the exact over-inclusion correction."""
    o = np.asarray(input).reshape(-1)[:G_REAL * D].astype(np.float32)
    t = np.asarray(gdt_ts).reshape(-1)[:G_REAL * D].astype(np.float32)

    t_g = np.zeros((G_PAD, D), np.float32)
    o_g = np.zeros((G_PAD, D), np.float32)
    t_g[:G_REAL] = t.reshape(G_REAL, D)
    o_g[:G_REAL] = o.reshape(G_REAL, D)

    idx = np.argsort(t_g, axis=1, kind="stable")
    t_g = np.take_along_axis(t_g, idx, axis=1)
    o_g = np.take_along_axis(o_g, idx, axis=1)

    # c[g, i] = #{j : t_gj < t_gi - 0.1}; rows are sorted so this is a prefix.
    c = np.empty((G_PAD, D), np.int64)
    for g in range(G_PAD):
        c[g] = np.searchsorted(t_g[g], t_g[g] - THRESHOLD, side="left")

    a_g = (1.0 + o_g).astype(np.float32)
    a_g[G_REAL:] = PAD_A

    cw = c.reshape(G_PAD, N_SLICES, WIN)
    J = cw.max(axis=(0, 2))
    J = np.minimum(J, D).astype(np.int64)

    # ---- exact over-inclusion correction ----
    Jrow = np.broadcast_to(J[:, None], (N_SLICES, WIN)).reshape(D)
    cr = c.copy()
    cr[G_REAL:] = D  # pad rows contribute exactly 0 — skip
    width = int((Jrow[None, :] - cr).max())
    corr = np.float64(0.0)
    for w in range(max(width, 0)):
        j = cr + w
        live = j < Jrow[None, :]
        jj = np.minimum(j, D - 1)
        v = a_g - o_g[np.arange(G_PAD)[:, None], jj]
        corr += np.where(live, np.maximum(v, 0.0), 0.0).sum(dtype=np.float64)

    # ---- per-core device arrays ----
    bf = ml_dtypes.bfloat16
    s_idx = np.arange(N_SLICES)
    k_idx = np.arange(M)
    p_idx = np.arange(P)
    i_map = (WIN * s_idx[:, None, None] + 2 * k_idx[None, :, None]
             + (p_idx % 2)[None, None, :])          # [S, M, P]
    gind = (p_idx[None, :] // 2 == np.arange(GPC)[:, None])  # [GPC, P]

    in_maps = []
    for cidx in range(N_CORES):
        gsl = slice(cidx * GPC, (cidx + 1) * GPC)
        a_core = a_g[gsl]
        o_core = o_g[gsl]
        g_of_p = p_idx // 2
        a_rows = a_core[g_of_p[None, None, :], i_map]    # [S, M, P]
        a_rows = a_rows.transpose(1, 0, 2).reshape(M, N_SLICES * P)

        rhs = np.zeros((KDIM, RHS_W), np.float32)
        rhs[:GPC].reshape(GPC, D, M)[:] = -o_core[:, :, None]
        for k in range(M):
            rhs[GPC + k, k::M] = 1.0

        w_full = np.zeros((KDIM, N_SLICES * P), np.float32)
        w_full[:GPC] = np.tile(gind, (1, N_SLICES))
        w_full[GPC:] = a_rows
        wbf = w_full.astype(bf)

        im = {"rhs": rhs.astype(bf)}
        active = [s for s in range(N_SLICES) if J[s] > 0]
        CS = 8
        chunks = [active[i:i + CS] for i in range(0, len(active), CS)]
        for ch, sl in enumerate(chunks):
            im[f"w{ch}"] = np.ascontiguousarray(
                np.concatenate([wbf[:, s * P:(s + 1) * P] for s in sl],
                               axis=1))
        in_maps.append(im)

    return in_maps, J, corr


def kernel(input, gdt_ts):
    in_maps, J, corr = _host_prep(input, gdt_ts)

    key = tuple(int(x) for x in J)
    if key not in _CACHED:
        _CACHED[key] = _build_program(J)
    nc, tiles = _CACHED[key]

    res = run_bass_kernel_spmd(nc, in_maps, list(range(N_CORES)))

    total = np.float64(0.0)
    for cidx in range(N_CORES):
        total += res.results[cidx]["acc_a"].astype(np.float64).sum()
        total += res.results[cidx]["acc_d"].astype(np.float64).sum()
        total += res.results[cidx]["accl_a"].astype(np.float64).sum()
        total += res.results[cidx]["accl_d"].astype(np.float64).sum()

    loss = 2.0 * (total - corr) / float(N_PAIRS)
    return np.array([loss], dtype=np.float32)
